# revision 36
# baseline (speedup 1.0000x reference)
"""Trainium2 Bass kernel for nn_ConfounderStackLayers.

Computation (per batch row b, confounder c):
    h0 = relu(x @ W0[c].T + b0[c])        # [B, H0]
    h1 = relu(h0 @ W1[c].T + b1[c])       # [B, H1]
    out[b, c] = h1 @ W2[c, 0] + b2[c]     # scalar head

Sharding: data-parallel over batch across 8 cores (2048 rows each), weights
replicated, no collectives.

On-device layout is "feature-major" (everything transposed) so the matmul
contraction dim is always the partition dim and no on-chip transposes are
needed:
    layer0: out0[h, b] = sum_d W0T[d, h] * xT[d, b]     (lhsT=W0T, rhs=xT)
    layer1: out1[o, b] = sum_i W1T[i, o] * h0[i, b]     (lhsT=W1T, rhs=h0)
    layer2: out2[c, b] = sum_k W2blk[k, c] * h1[k, b]   (block-diag lhsT)
Host pre-transposes x and the weights into these layouts (host time is not
part of the graded HW execution time).

This walrus build enforces AT MOST ONE semaphore wait per instruction
("Too many sync wait commands" otherwise), and Tile emits a wait for every
dependency sem the engine has not yet observed — including same-engine
waits for pool-slot reuse. The kernel therefore keeps every instruction to
a single unobserved sem:
  - tiny single-wait "touch" ops pre-observe DMA-completion sems and
    cross-engine sems before the real consumer runs;
  - each h0/h1 tile is written by exactly one engine (alternating per c,
    so slot reuse lands on the same engine's sem);
  - every eviction is preceded by a same-engine touch that reads one
    element of the psum tile (absorbing the PE wait), leaving the evict
    with only its slot-release self-wait;
  - touch/evict order is pinned with no-sync scheduler edges;
  - DMA count is kept low (4 input HWDGE lanes + 1 output SWDGE lane) so
    the kernel-tail drain stays within its wait budget.
"""

import os
from contextlib import ExitStack

import numpy as np

import concourse.bass as bass
import concourse.mybir as mybir
import concourse.tile as tile
from concourse.tile_rust import add_dep_helper
from concourse.bass_utils import run_bass_kernel_spmd

NCORES = 8
B, C, D, H0, H1 = 16384, 8, 256, 512, 256
BS = B // NCORES          # 2048 batch rows per core
BC = 512                  # batch chunk (one psum bank of fp32)
NB = BS // BC             # 4
KT0, MT0 = D // 128, H0 // 128    # 2, 4
KT1, MT1 = H0 // 128, H1 // 128   # 4, 2
KT2 = H1 // 128                   # 2

WCOLS = KT0 * H0 + KT1 * H1       # per-c combined weight columns (w0 then w1)
W2_COLS = C * KT2 * C             # 128
B0_OFF, B1_OFF, B2_OFF = 0, C * MT0, C * MT0 + C * MT1
BIAS_COLS = B2_OFF + 1            # 49

# matmul operand mode: "f32" (exact, 4 cyc/row), "f32r" (reduced-precision
# fp32 path, 1 cyc/row at N>=256), "bf16" (1 cyc/row, halves DMA traffic)
MM_MODE = os.environ.get("KERNEL_MM_MODE", "f32r")

_CACHE = {}


def _build(mm: str) -> bass.Bass:
    f32 = mybir.dt.float32
    if mm == "bf16":
        wdt = mybir.dt.bfloat16
    elif mm == "f32r":
        wdt = mybir.dt.float32r
    else:
        wdt = f32
    relu = mybir.ActivationFunctionType.Relu
    copy_f = mybir.ActivationFunctionType.Copy
    add_op = mybir.AluOpType.add
    max_op = mybir.AluOpType.max

    nc = bass.Bass(trn_type="TRN2")
    xd = nc.dram_tensor("xd", [128, NB, KT0, BC], wdt, kind="ExternalInput")
    w2d = nc.dram_tensor("w2d", [128, W2_COLS], wdt, kind="ExternalInput")
    wcat = nc.dram_tensor("wcat", [C, 128, WCOLS], wdt, kind="ExternalInput")
    biasd = nc.dram_tensor("biasd", [128, BIAS_COLS], f32, kind="ExternalInput")
    outT = nc.dram_tensor("outT", [C, BS], f32, kind="ExternalOutput")

    with tile.TileContext(nc) as tc, ExitStack() as ctx:
        consts = ctx.enter_context(tc.tile_pool(name="consts", bufs=1))
        h0p = ctx.enter_context(tc.tile_pool(name="h0", bufs=2))
        h1p = ctx.enter_context(tc.tile_pool(name="h1", bufs=2))
        ps0p = ctx.enter_context(tc.tile_pool(name="ps0", bufs=3, space="PSUM"))
        ps1p = ctx.enter_context(tc.tile_pool(name="ps1", bufs=2, space="PSUM"))
        ps2p = ctx.enter_context(tc.tile_pool(name="ps2", bufs=2, space="PSUM"))
        pewp = ctx.enter_context(tc.tile_pool(name="pew", bufs=1, space="PSUM"))

        # HWDGE DMAs drain FIFO per ring, so issue in consumption order:
        # the small w2/bias blocks, then x and per-c weights interleaved so
        # the first layer-0 matmul only waits ~1.3 MB, and each c's weights
        # land while the previous c computes.
        funnel_deps = []
        w2s = consts.tile([128, W2_COLS], wdt, tag="w2s")
        funnel_deps.append(nc.scalar.dma_start(out=w2s, in_=w2d[:, :]))
        bias = consts.tile([128, BIAS_COLS], f32, tag="bias")
        funnel_deps.append(nc.scalar.dma_start(out=bias, in_=biasd[:, :]))
        xs = consts.tile([128, NB, KT0, BC], wdt, tag="xs")
        wts = []
        for c in range(C):
            wc_tile = consts.tile([128, WCOLS], wdt, tag=f"w_{c}")
            wts.append(wc_tile)
        W0C = KT0 * H0  # first WCOLS half = layer-0 weights, rest = layer-1
        dma_order = [("x", 0), ("w0", 0), ("w1", 0), ("w0", 1), ("w1", 1),
                     ("x", 1), ("w0", 2), ("w1", 2), ("w0", 3), ("w1", 3),
                     ("x", 2), ("w0", 4), ("w1", 4), ("w0", 5), ("w1", 5),
                     ("x", 3), ("w0", 6), ("w1", 6), ("w0", 7), ("w1", 7)]
        for kind, i in dma_order:
            # The first chunk (bi=0 x, c=0 weights) goes out on the ACT
            # HWDGE ring, which starts issuing descriptors well before the
            # Sync ring clears its prologue.
            eng = nc.scalar if (kind, i) in (("x", 0), ("w0", 0), ("w1", 0)) else nc.sync
            if kind == "x":
                funnel_deps.append(
                    eng.dma_start(out=xs[:, i], in_=xd[:, i])
                )
            elif kind == "w0":
                funnel_deps.append(
                    eng.dma_start(out=wts[i][:, 0:W0C], in_=wcat[i][:, 0:W0C])
                )
            else:
                funnel_deps.append(
                    eng.dma_start(
                        out=wts[i][:, W0C:WCOLS], in_=wcat[i][:, W0C:WCOLS]
                    )
                )

        def xs_ap(kt, bi):
            return xs[:, bi, kt, :]

        def w2_ap(t):
            return w2s[:, t * C:(t + 1) * C]

        def w0_ap(c, kt, mt):
            lo = kt * H0 + mt * 128
            return wts[c][:, lo:lo + 128]

        def w1_ap(c, kt, mt):
            lo = KT0 * H0 + kt * H1 + mt * 128
            return wts[c][:, lo:lo + 128]

        def b0_ap(c, mt):
            return bias[:, B0_OFF + c * MT0 + mt:B0_OFF + c * MT0 + mt + 1]

        def b1_ap(c, mt):
            return bias[:, B1_OFF + c * MT1 + mt:B1_OFF + c * MT1 + mt + 1]

        b2_ap = bias[0:C, B2_OFF:B2_OFF + 1]

        # Per-engine scratch: unique column per touch so no touch ever
        # overlaps another (no WAW deps between touches at all).
        act_scr = consts.tile([1, 256], f32, tag="act_scr")
        dve_scr = consts.tile([1, 256], f32, tag="dve_scr")
        outt = consts.tile([C, NB, BC], f32, tag="outt")
        pewarm = pewp.tile([C, BC], f32, tag="pewarm")
        cnt = {"act": 0, "dve": 0, "pe": 0}

        def act_touch(src_ap):
            j = cnt["act"]; cnt["act"] += 1
            return nc.scalar.activation(act_scr[0:1, j:j + 1], src_ap, copy_f)

        def dve_touch(src_ap):
            j = cnt["dve"]; cnt["dve"] += 1
            return nc.vector.tensor_copy(dve_scr[0:1, j:j + 1], src_ap)

        def pe_touch(col_ap):
            # col_ap: [P, 1] SBUF column; writes 1 garbage element into the
            # never-read pewarm bank. Absorbs exactly one sem wait on PE.
            # f32r APs are bitcast to f32: tiny-N matmuls trip the ISA's
            # fp32r restrictions, and the result is garbage anyway.
            j = cnt["pe"]; cnt["pe"] += 1
            if col_ap.dtype == mybir.dt.float32r:
                col_ap = col_ap.bitcast(mybir.dt.float32)
            return nc.tensor.matmul(
                pewarm[0:1, j:j + 1], lhsT=col_ap, rhs=col_ap,
                start=True, stop=True,
            )

        # HAM pre-warm: dependency-free fp32 matmuls keep the PE array busy
        # while the input DMAs land, flipping the clock gate to 8/8 (2.4
        # GHz) before the first real matmul instead of ~10us into the run.
        wscr = consts.tile([128, 128], f32, tag="wscr")
        warm_ms = nc.vector.memset(wscr, 1.0)
        for i in range(16):
            wu = nc.tensor.matmul(
                pewarm[0:1, 0:128], lhsT=wscr[:, 0:1], rhs=wscr[:, 0:128],
                start=True, stop=True,
            )
            if i == 0:
                add_dep_helper(wu.ins, warm_ms.ins, False, "memset before warmups")

        # Pre-observe the bias DMA lane on both evict engines, and the w2
        # DMA lane on PE (the opener consumes w2s first).
        act_bias_touch = act_touch(bias[0:1, 0:1])
        dve_bias_touch = dve_touch(bias[0:1, 0:1])
        pe_w2s_touch = pe_touch(w2s[:, 0:1])

        first_evict = {"act": True, "dve": True}
        last_inst = {"pe": None, "dve": None}

        def evict(engine, dst_ap, src_ps, bias_ap, with_relu=True):
            # Touch first (absorbs the PE-completion wait), then the real
            # evict carries only its slot-release self-wait.
            if engine == "act":
                t = act_touch(src_ps[0:1, 0:1])
                if with_relu:
                    e = nc.scalar.activation(dst_ap, src_ps, relu, bias=bias_ap)
                else:
                    e = nc.scalar.add(dst_ap, src_ps, bias_ap)
            else:
                t = dve_touch(src_ps[0:1, 0:1])
                e = nc.vector.tensor_scalar(
                    dst_ap, src_ps, bias_ap, 0.0, add_op, max_op
                )
            add_dep_helper(e.ins, t.ins, False, "touch before evict")
            if engine == "dve":
                last_inst["dve"] = e
            if first_evict[engine]:
                first_evict[engine] = False
                bt = act_bias_touch if engine == "act" else dve_bias_touch
                add_dep_helper(t.ins, bt.ins, False, "bias touch first")
            return e

        for bi in range(NB):
            if bi >= 2:
                # PE re-acquires the ps2 psum slot: observe the ACT
                # out-evict that last read it, then let the opener matmul
                # absorb the slot-release wait.
                ot = pe_touch(outt[:, bi - 2, 0:1])
            ps2 = ps2p.tile([C, BC], f32)
            op_w = w2_ap(0)
            if op_w.dtype == mybir.dt.float32r:
                op_w = op_w.bitcast(mybir.dt.float32)
            opener = nc.tensor.matmul(
                ps2[:, 0:C], lhsT=op_w, rhs=op_w,
                start=True, stop=True,
            )
            add_dep_helper(opener.ins, (ot if bi >= 2 else pe_w2s_touch).ins,
                           False, "observe before opener")
            # Observe this bi's x-chunk DMA lane before its first matmul.
            xt = pe_touch(xs[:, bi, 0, 0:1])
            add_dep_helper(xt.ins, opener.ins, False, "opener before x touch")
            for c in range(C):
                e0, e1 = ("act", "dve") if c % 2 == 0 else ("dve", "act")
                if bi == 0:
                    wt = pe_touch(wts[c][:, 0:1])
                h0 = h0p.tile([128, KT1, BC], wdt)
                for mt in range(MT0):
                    ps0 = ps0p.tile([128, BC], f32)
                    for kt in range(KT0):
                        mmi = nc.tensor.matmul(
                            ps0,
                            lhsT=w0_ap(c, kt, mt),
                            rhs=xs_ap(kt, bi),
                            start=(kt == 0),
                            stop=(kt == KT0 - 1),
                        )
                        if bi == 0 and mt == 0 and kt == 0:
                            add_dep_helper(mmi.ins, wt.ins, False, "wt touch first")
                        if c == 0 and mt == 0 and kt == 0:
                            add_dep_helper(mmi.ins, xt.ins, False, "x touch first")
                    evict(e0, h0[:, mt, :], ps0, b0_ap(c, mt))
                if bi == 0:
                    wt1 = pe_touch(wts[c][:, W0C:W0C + 1])
                h1 = h1p.tile([128, KT2, BC], wdt)
                for mt in range(MT1):
                    ps1 = ps1p.tile([128, BC], f32)
                    for kt in range(KT1):
                        mm1 = nc.tensor.matmul(
                            ps1,
                            lhsT=w1_ap(c, kt, mt),
                            rhs=h0[:, kt, :],
                            start=(kt == 0),
                            stop=(kt == KT1 - 1),
                        )
                        if bi == 0 and mt == 0 and kt == 0:
                            add_dep_helper(mm1.ins, wt1.ins, False, "w1 touch first")
                    evict(e1, h1[:, mt, :], ps1, b1_ap(c, mt))
                for kt in range(KT2):
                    last_inst["pe"] = nc.tensor.matmul(
                        ps2,
                        lhsT=w2_ap(c * KT2 + kt),
                        rhs=h1[:, kt, :],
                        start=(c == 0 and kt == 0),
                        stop=(c == C - 1 and kt == KT2 - 1),
                    )
            out_ev = evict("act", outt[:, bi, :], ps2, b2_ap, with_relu=False)
        # Single output DMA on the (otherwise empty) SWDGE ring: one wait
        # (ACT wrote outt) and one extra drain sem total.
        out_dma = nc.gpsimd.dma_start(
            out=outT[:, :], in_=outt.rearrange("c nb bc -> c (nb bc)")
        )
        # Funnel: the kernel-tail drain would otherwise wait on every DMA
        # lane + engine sem at once (way over the 1-wait-per-instruction
        # limit). A chain of single-wait SP nops observes each outstanding
        # sem first, leaving the drain nothing new to wait on.
        funnel_deps += [out_dma, out_ev, last_inst["pe"], last_inst["dve"]]
        for dep in funnel_deps:
            n = nc.sync.nop()
            add_dep_helper(n.ins, dep.ins, True, "drain funnel")
    return nc


def _np_wdt(mm: str):
    if mm == "bf16":
        import ml_dtypes

        return ml_dtypes.bfloat16
    return np.float32


def kernel(x, W0, b0, W1, b1, W2, b2, trace=False):
    mm = MM_MODE
    key = ("nc", mm)
    if key not in _CACHE:
        _CACHE[key] = _build(mm)
    nc = _CACHE[key]
    wnp = _np_wdt(mm)

    x = np.ascontiguousarray(np.asarray(x, dtype=np.float32))
    W0 = np.asarray(W0, dtype=np.float32)
    W1 = np.asarray(W1, dtype=np.float32)
    W2 = np.asarray(W2, dtype=np.float32)
    b0 = np.asarray(b0, dtype=np.float32)
    b1 = np.asarray(b1, dtype=np.float32)
    b2 = np.asarray(b2, dtype=np.float32)

    # Combined per-c weight block: [C, 128, KT0*H0 + KT1*H1] where
    # wcat[c, p, kt*H0 + h] = W0[c, h, kt*128+p] and
    # wcat[c, p, KT0*H0 + kt*H1 + o] = W1[c, o, kt*128+p].
    wcat = np.empty((C, 128, WCOLS), dtype=np.float32)
    w0v = wcat[:, :, :KT0 * H0].reshape(C, 128, KT0, H0)
    w0v[...] = W0.reshape(C, H0, KT0, 128).transpose(0, 3, 2, 1)
    w1v = wcat[:, :, KT0 * H0:].reshape(C, 128, KT1, H1)
    w1v[...] = W1.reshape(C, H1, KT1, 128).transpose(0, 3, 2, 1)
    wcat = np.ascontiguousarray(wcat).astype(wnp)

    # Layer-2 block-diagonal lhsT tiles:
    # w2part[p, t*C + cc] = W2[c, 0, i] iff (c, i) == (t // KT2, (t % KT2)*128 + p)
    w2part = np.zeros((128, W2_COLS), dtype=np.float32)
    for c in range(C):
        for kt in range(KT2):
            t = c * KT2 + kt
            w2part[:, t * C + c] = W2[c, 0, kt * 128:(kt + 1) * 128]
    w2part = w2part.astype(wnp)

    biasd = np.zeros((128, BIAS_COLS), dtype=np.float32)
    biasd[:, B0_OFF:B0_OFF + C * MT0] = (
        b0.reshape(C, MT0, 128).transpose(2, 0, 1).reshape(128, C * MT0)
    )
    biasd[:, B1_OFF:B1_OFF + C * MT1] = (
        b1.reshape(C, MT1, 128).transpose(2, 0, 1).reshape(128, C * MT1)
    )
    biasd[0:C, B2_OFF] = b2

    xTfull = np.ascontiguousarray(x.T)  # [D, B] fp32
    in_maps = []
    for s in range(NCORES):
        xsh = xTfull[:, s * BS:(s + 1) * BS]          # [D, BS]
        # xd[p, bi, kt, b] = x[s*BS + bi*BC + b, kt*128 + p]
        xd = np.ascontiguousarray(
            xsh.reshape(KT0, 128, NB, BC).transpose(1, 2, 0, 3)
        ).astype(wnp)
        in_maps.append(
            {
                "xd": xd,
                "w2d": w2part,
                "wcat": wcat,
                "biasd": biasd,
            }
        )

    res = run_bass_kernel_spmd(
        nc, in_maps, core_ids=list(range(NCORES)), trace=trace
    )
    _CACHE["last_result"] = res

    out = np.empty((B, C), dtype=np.float32)
    for s in range(NCORES):
        out[s * BS:(s + 1) * BS, :] = res.results[s]["outT"].T
    return out


# revision 37
# speedup vs baseline: 1.0453x; 1.0453x over previous
"""Trainium2 Bass kernel for nn_ConfounderStackLayers.

Computation (per batch row b, confounder c):
    h0 = relu(x @ W0[c].T + b0[c])        # [B, H0]
    h1 = relu(h0 @ W1[c].T + b1[c])       # [B, H1]
    out[b, c] = h1 @ W2[c, 0] + b2[c]     # scalar head

Sharding: data-parallel over batch across 8 cores (2048 rows each), weights
replicated, no collectives.

On-device layout is "feature-major" (everything transposed) so the matmul
contraction dim is always the partition dim and no on-chip transposes are
needed:
    layer0: out0[h, b] = sum_d W0T[d, h] * xT[d, b]     (lhsT=W0T, rhs=xT)
    layer1: out1[o, b] = sum_i W1T[i, o] * h0[i, b]     (lhsT=W1T, rhs=h0)
    layer2: out2[c, b] = sum_k W2blk[k, c] * h1[k, b]   (block-diag lhsT)
Host pre-transposes x and the weights into these layouts (host time is not
part of the graded HW execution time).

This walrus build enforces AT MOST ONE semaphore wait per instruction
("Too many sync wait commands" otherwise), and Tile emits a wait for every
dependency sem the engine has not yet observed — including same-engine
waits for pool-slot reuse. The kernel therefore keeps every instruction to
a single unobserved sem:
  - tiny single-wait "touch" ops pre-observe DMA-completion sems and
    cross-engine sems before the real consumer runs;
  - each h0/h1 tile is written by exactly one engine (alternating per c,
    so slot reuse lands on the same engine's sem);
  - every eviction is preceded by a same-engine touch that reads one
    element of the psum tile (absorbing the PE wait), leaving the evict
    with only its slot-release self-wait;
  - touch/evict order is pinned with no-sync scheduler edges;
  - DMA count is kept low (4 input HWDGE lanes + 1 output SWDGE lane) so
    the kernel-tail drain stays within its wait budget.
"""

import os
from contextlib import ExitStack

import numpy as np

import concourse.bass as bass
import concourse.mybir as mybir
import concourse.tile as tile
from concourse.tile_rust import add_dep_helper
from concourse.bass_utils import run_bass_kernel_spmd

NCORES = 8
B, C, D, H0, H1 = 16384, 8, 256, 512, 256
BS = B // NCORES          # 2048 batch rows per core
BC = 512                  # batch chunk (one psum bank of fp32)
NB = BS // BC             # 4
KT0, MT0 = D // 128, H0 // 128    # 2, 4
KT1, MT1 = H0 // 128, H1 // 128   # 4, 2
KT2 = H1 // 128                   # 2

WCOLS = KT0 * H0 + KT1 * H1       # per-c combined weight columns (w0 then w1)
W2_COLS = C * KT2 * C             # 128
B0_OFF, B1_OFF, B2_OFF = 0, C * MT0, C * MT0 + C * MT1
BIAS_COLS = B2_OFF + 1            # 49

# matmul operand mode: "f32" (exact, 4 cyc/row), "f32r" (reduced-precision
# fp32 path, 1 cyc/row at N>=256), "bf16" (1 cyc/row, halves DMA traffic)
MM_MODE = os.environ.get("KERNEL_MM_MODE", "f32r")

_CACHE = {}


def _build(mm: str) -> bass.Bass:
    f32 = mybir.dt.float32
    if mm == "bf16":
        wdt = mybir.dt.bfloat16
    elif mm == "f32r":
        wdt = mybir.dt.float32r
    else:
        wdt = f32
    relu = mybir.ActivationFunctionType.Relu
    copy_f = mybir.ActivationFunctionType.Copy
    add_op = mybir.AluOpType.add
    max_op = mybir.AluOpType.max

    nc = bass.Bass(trn_type="TRN2")
    xd = nc.dram_tensor("xd", [128, NB, KT0, BC], wdt, kind="ExternalInput")
    w2d = nc.dram_tensor("w2d", [128, W2_COLS], wdt, kind="ExternalInput")
    wcat = nc.dram_tensor("wcat", [C, 128, WCOLS], wdt, kind="ExternalInput")
    biasd = nc.dram_tensor("biasd", [128, BIAS_COLS], f32, kind="ExternalInput")
    outT = nc.dram_tensor("outT", [C, BS], f32, kind="ExternalOutput")

    with tile.TileContext(nc) as tc, ExitStack() as ctx:
        consts = ctx.enter_context(tc.tile_pool(name="consts", bufs=1))
        h0p = ctx.enter_context(tc.tile_pool(name="h0", bufs=2))
        h1p = ctx.enter_context(tc.tile_pool(name="h1", bufs=2))
        ps0p = ctx.enter_context(tc.tile_pool(name="ps0", bufs=3, space="PSUM"))
        ps1p = ctx.enter_context(tc.tile_pool(name="ps1", bufs=2, space="PSUM"))
        ps2p = ctx.enter_context(tc.tile_pool(name="ps2", bufs=2, space="PSUM"))
        pewp = ctx.enter_context(tc.tile_pool(name="pew", bufs=1, space="PSUM"))

        # HWDGE DMAs drain FIFO per ring, so issue in consumption order:
        # the small w2/bias blocks, then x and per-c weights interleaved so
        # the first layer-0 matmul only waits ~1.3 MB, and each c's weights
        # land while the previous c computes.
        funnel_deps = []
        w2s = consts.tile([128, W2_COLS], wdt, tag="w2s")
        funnel_deps.append(nc.scalar.dma_start(out=w2s, in_=w2d[:, :]))
        bias = consts.tile([128, BIAS_COLS], f32, tag="bias")
        funnel_deps.append(nc.scalar.dma_start(out=bias, in_=biasd[:, :]))
        xs = consts.tile([128, NB, KT0, BC], wdt, tag="xs")
        wts = []
        for c in range(C):
            wc_tile = consts.tile([128, WCOLS], wdt, tag=f"w_{c}")
            wts.append(wc_tile)
        W0C = KT0 * H0  # first WCOLS half = layer-0 weights, rest = layer-1
        dma_order = [("x", 0), ("w0", 0), ("w1", 0), ("w0", 1), ("w1", 1),
                     ("x", 1), ("w0", 2), ("w1", 2), ("w0", 3), ("w1", 3),
                     ("x", 2), ("w0", 4), ("w1", 4), ("w0", 5), ("w1", 5),
                     ("x", 3), ("w0", 6), ("w1", 6), ("w0", 7), ("w1", 7)]
        for kind, i in dma_order:
            # The first chunk (bi=0 x, c=0 weights) goes out on the ACT
            # HWDGE ring, which starts issuing descriptors well before the
            # Sync ring clears its prologue.
            eng = nc.scalar if (kind, i) in (("x", 0), ("w0", 0), ("w1", 0)) else nc.sync
            if kind == "x":
                funnel_deps.append(
                    eng.dma_start(out=xs[:, i], in_=xd[:, i])
                )
            elif kind == "w0":
                funnel_deps.append(
                    eng.dma_start(out=wts[i][:, 0:W0C], in_=wcat[i][:, 0:W0C])
                )
            else:
                funnel_deps.append(
                    eng.dma_start(
                        out=wts[i][:, W0C:WCOLS], in_=wcat[i][:, W0C:WCOLS]
                    )
                )

        def xs_ap(kt, bi):
            return xs[:, bi, kt, :]

        def w2_ap(t):
            return w2s[:, t * C:(t + 1) * C]

        def w0_ap(c, kt, mt):
            lo = kt * H0 + mt * 128
            return wts[c][:, lo:lo + 128]

        def w1_ap(c, kt, mt):
            lo = KT0 * H0 + kt * H1 + mt * 128
            return wts[c][:, lo:lo + 128]

        def b0_ap(c, mt):
            return bias[:, B0_OFF + c * MT0 + mt:B0_OFF + c * MT0 + mt + 1]

        def b1_ap(c, mt):
            return bias[:, B1_OFF + c * MT1 + mt:B1_OFF + c * MT1 + mt + 1]

        b2_ap = bias[0:C, B2_OFF:B2_OFF + 1]

        # Per-engine scratch: unique column per touch so no touch ever
        # overlaps another (no WAW deps between touches at all).
        act_scr = consts.tile([1, 256], f32, tag="act_scr")
        dve_scr = consts.tile([1, 256], f32, tag="dve_scr")
        outt = consts.tile([C, NB, BC], f32, tag="outt")
        pewarm = pewp.tile([C, BC], f32, tag="pewarm")
        cnt = {"act": 0, "dve": 0, "pe": 0}

        def act_touch(src_ap):
            j = cnt["act"]; cnt["act"] += 1
            return nc.scalar.activation(act_scr[0:1, j:j + 1], src_ap, copy_f)

        def dve_touch(src_ap):
            j = cnt["dve"]; cnt["dve"] += 1
            return nc.vector.tensor_copy(dve_scr[0:1, j:j + 1], src_ap)

        def pe_touch(col_ap):
            # col_ap: [P, 1] SBUF column; writes 1 garbage element into the
            # never-read pewarm bank. Absorbs exactly one sem wait on PE.
            # f32r APs are bitcast to f32: tiny-N matmuls trip the ISA's
            # fp32r restrictions, and the result is garbage anyway.
            j = cnt["pe"]; cnt["pe"] += 1
            if col_ap.dtype == mybir.dt.float32r:
                col_ap = col_ap.bitcast(mybir.dt.float32)
            return nc.tensor.matmul(
                pewarm[0:1, j:j + 1], lhsT=col_ap, rhs=col_ap,
                start=True, stop=True,
            )

        # Pre-observe the bias DMA lane on both evict engines, and the w2
        # DMA lane on PE (the opener consumes w2s first).
        act_bias_touch = act_touch(bias[0:1, 0:1])
        dve_bias_touch = dve_touch(bias[0:1, 0:1])
        pe_w2s_touch = pe_touch(w2s[:, 0:1])

        first_evict = {"act": True, "dve": True}
        last_inst = {"pe": None, "dve": None}

        def evict(engine, dst_ap, src_ps, bias_ap, with_relu=True):
            # Touch first (absorbs the PE-completion wait), then the real
            # evict carries only its slot-release self-wait.
            if engine == "act":
                t = act_touch(src_ps[0:1, 0:1])
                if with_relu:
                    e = nc.scalar.activation(dst_ap, src_ps, relu, bias=bias_ap)
                else:
                    e = nc.scalar.add(dst_ap, src_ps, bias_ap)
            else:
                t = dve_touch(src_ps[0:1, 0:1])
                e = nc.vector.tensor_scalar(
                    dst_ap, src_ps, bias_ap, 0.0, add_op, max_op
                )
            add_dep_helper(e.ins, t.ins, False, "touch before evict")
            if engine == "dve":
                last_inst["dve"] = e
            if first_evict[engine]:
                first_evict[engine] = False
                bt = act_bias_touch if engine == "act" else dve_bias_touch
                add_dep_helper(t.ins, bt.ins, False, "bias touch first")
            return e

        for bi in range(NB):
            if bi >= 2:
                # PE re-acquires the ps2 psum slot: observe the ACT
                # out-evict that last read it, then let the opener matmul
                # absorb the slot-release wait.
                ot = pe_touch(outt[:, bi - 2, 0:1])
            ps2 = ps2p.tile([C, BC], f32)
            op_w = w2_ap(0)
            if op_w.dtype == mybir.dt.float32r:
                op_w = op_w.bitcast(mybir.dt.float32)
            opener = nc.tensor.matmul(
                ps2[:, 0:C], lhsT=op_w, rhs=op_w,
                start=True, stop=True,
            )
            add_dep_helper(opener.ins, (ot if bi >= 2 else pe_w2s_touch).ins,
                           False, "observe before opener")
            # Observe this bi's x-chunk DMA lane before its first matmul.
            xt = pe_touch(xs[:, bi, 0, 0:1])
            add_dep_helper(xt.ins, opener.ins, False, "opener before x touch")
            for c in range(C):
                e0, e1 = ("act", "dve") if c % 2 == 0 else ("dve", "act")
                if bi == 0:
                    wt = pe_touch(wts[c][:, 0:1])
                h0 = h0p.tile([128, KT1, BC], wdt)
                for mt in range(MT0):
                    ps0 = ps0p.tile([128, BC], f32)
                    for kt in range(KT0):
                        mmi = nc.tensor.matmul(
                            ps0,
                            lhsT=w0_ap(c, kt, mt),
                            rhs=xs_ap(kt, bi),
                            start=(kt == 0),
                            stop=(kt == KT0 - 1),
                        )
                        if bi == 0 and mt == 0 and kt == 0:
                            add_dep_helper(mmi.ins, wt.ins, False, "wt touch first")
                        if c == 0 and mt == 0 and kt == 0:
                            add_dep_helper(mmi.ins, xt.ins, False, "x touch first")
                    evict(e0, h0[:, mt, :], ps0, b0_ap(c, mt))
                if bi == 0:
                    wt1 = pe_touch(wts[c][:, W0C:W0C + 1])
                h1 = h1p.tile([128, KT2, BC], wdt)
                for mt in range(MT1):
                    ps1 = ps1p.tile([128, BC], f32)
                    for kt in range(KT1):
                        mm1 = nc.tensor.matmul(
                            ps1,
                            lhsT=w1_ap(c, kt, mt),
                            rhs=h0[:, kt, :],
                            start=(kt == 0),
                            stop=(kt == KT1 - 1),
                        )
                        if bi == 0 and mt == 0 and kt == 0:
                            add_dep_helper(mm1.ins, wt1.ins, False, "w1 touch first")
                    evict(e1, h1[:, mt, :], ps1, b1_ap(c, mt))
                for kt in range(KT2):
                    last_inst["pe"] = nc.tensor.matmul(
                        ps2,
                        lhsT=w2_ap(c * KT2 + kt),
                        rhs=h1[:, kt, :],
                        start=(c == 0 and kt == 0),
                        stop=(c == C - 1 and kt == KT2 - 1),
                    )
            out_ev = evict("act", outt[:, bi, :], ps2, b2_ap, with_relu=False)
        # Single output DMA on the (otherwise empty) SWDGE ring: one wait
        # (ACT wrote outt) and one extra drain sem total.
        out_dma = nc.gpsimd.dma_start(
            out=outT[:, :], in_=outt.rearrange("c nb bc -> c (nb bc)")
        )
        # Funnel: the kernel-tail drain would otherwise wait on every DMA
        # lane + engine sem at once (way over the 1-wait-per-instruction
        # limit). A chain of single-wait SP nops observes each outstanding
        # sem first, leaving the drain nothing new to wait on.
        funnel_deps += [out_dma, out_ev, last_inst["pe"], last_inst["dve"]]
        for dep in funnel_deps:
            n = nc.sync.nop()
            add_dep_helper(n.ins, dep.ins, True, "drain funnel")
    return nc


def _np_wdt(mm: str):
    if mm == "bf16":
        import ml_dtypes

        return ml_dtypes.bfloat16
    return np.float32


def kernel(x, W0, b0, W1, b1, W2, b2, trace=False):
    mm = MM_MODE
    key = ("nc", mm)
    if key not in _CACHE:
        _CACHE[key] = _build(mm)
    nc = _CACHE[key]
    wnp = _np_wdt(mm)

    x = np.ascontiguousarray(np.asarray(x, dtype=np.float32))
    W0 = np.asarray(W0, dtype=np.float32)
    W1 = np.asarray(W1, dtype=np.float32)
    W2 = np.asarray(W2, dtype=np.float32)
    b0 = np.asarray(b0, dtype=np.float32)
    b1 = np.asarray(b1, dtype=np.float32)
    b2 = np.asarray(b2, dtype=np.float32)

    # Combined per-c weight block: [C, 128, KT0*H0 + KT1*H1] where
    # wcat[c, p, kt*H0 + h] = W0[c, h, kt*128+p] and
    # wcat[c, p, KT0*H0 + kt*H1 + o] = W1[c, o, kt*128+p].
    wcat = np.empty((C, 128, WCOLS), dtype=np.float32)
    w0v = wcat[:, :, :KT0 * H0].reshape(C, 128, KT0, H0)
    w0v[...] = W0.reshape(C, H0, KT0, 128).transpose(0, 3, 2, 1)
    w1v = wcat[:, :, KT0 * H0:].reshape(C, 128, KT1, H1)
    w1v[...] = W1.reshape(C, H1, KT1, 128).transpose(0, 3, 2, 1)
    wcat = np.ascontiguousarray(wcat).astype(wnp)

    # Layer-2 block-diagonal lhsT tiles:
    # w2part[p, t*C + cc] = W2[c, 0, i] iff (c, i) == (t // KT2, (t % KT2)*128 + p)
    w2part = np.zeros((128, W2_COLS), dtype=np.float32)
    for c in range(C):
        for kt in range(KT2):
            t = c * KT2 + kt
            w2part[:, t * C + c] = W2[c, 0, kt * 128:(kt + 1) * 128]
    w2part = w2part.astype(wnp)

    biasd = np.zeros((128, BIAS_COLS), dtype=np.float32)
    biasd[:, B0_OFF:B0_OFF + C * MT0] = (
        b0.reshape(C, MT0, 128).transpose(2, 0, 1).reshape(128, C * MT0)
    )
    biasd[:, B1_OFF:B1_OFF + C * MT1] = (
        b1.reshape(C, MT1, 128).transpose(2, 0, 1).reshape(128, C * MT1)
    )
    biasd[0:C, B2_OFF] = b2

    xTfull = np.ascontiguousarray(x.T)  # [D, B] fp32
    in_maps = []
    for s in range(NCORES):
        xsh = xTfull[:, s * BS:(s + 1) * BS]          # [D, BS]
        # xd[p, bi, kt, b] = x[s*BS + bi*BC + b, kt*128 + p]
        xd = np.ascontiguousarray(
            xsh.reshape(KT0, 128, NB, BC).transpose(1, 2, 0, 3)
        ).astype(wnp)
        in_maps.append(
            {
                "xd": xd,
                "w2d": w2part,
                "wcat": wcat,
                "biasd": biasd,
            }
        )

    res = run_bass_kernel_spmd(
        nc, in_maps, core_ids=list(range(NCORES)), trace=trace
    )
    _CACHE["last_result"] = res

    out = np.empty((B, C), dtype=np.float32)
    for s in range(NCORES):
        out[s * BS:(s + 1) * BS, :] = res.results[s]["outT"].T
    return out


# revision 38
# speedup vs baseline: 1.0527x; 1.0071x over previous
"""Trainium2 Bass kernel for nn_ConfounderStackLayers.

Computation (per batch row b, confounder c):
    h0 = relu(x @ W0[c].T + b0[c])        # [B, H0]
    h1 = relu(h0 @ W1[c].T + b1[c])       # [B, H1]
    out[b, c] = h1 @ W2[c, 0] + b2[c]     # scalar head

Sharding: data-parallel over batch across 8 cores (2048 rows each), weights
replicated, no collectives.

On-device layout is "feature-major" (everything transposed) so the matmul
contraction dim is always the partition dim and no on-chip transposes are
needed:
    layer0: out0[h, b] = sum_d W0T[d, h] * xT[d, b]     (lhsT=W0T, rhs=xT)
    layer1: out1[o, b] = sum_i W1T[i, o] * h0[i, b]     (lhsT=W1T, rhs=h0)
    layer2: out2[c, b] = sum_k W2blk[k, c] * h1[k, b]   (block-diag lhsT)
Host pre-transposes x and the weights into these layouts (host time is not
part of the graded HW execution time).

This walrus build enforces AT MOST ONE semaphore wait per instruction
("Too many sync wait commands" otherwise), and Tile emits a wait for every
dependency sem the engine has not yet observed — including same-engine
waits for pool-slot reuse. The kernel therefore keeps every instruction to
a single unobserved sem:
  - tiny single-wait "touch" ops pre-observe DMA-completion sems and
    cross-engine sems before the real consumer runs;
  - each h0/h1 tile is written by exactly one engine (alternating per c,
    so slot reuse lands on the same engine's sem);
  - every eviction is preceded by a same-engine touch that reads one
    element of the psum tile (absorbing the PE wait), leaving the evict
    with only its slot-release self-wait;
  - touch/evict order is pinned with no-sync scheduler edges;
  - DMA count is kept low (4 input HWDGE lanes + 1 output SWDGE lane) so
    the kernel-tail drain stays within its wait budget.
"""

import os
from contextlib import ExitStack

import numpy as np

import concourse.bass as bass
import concourse.mybir as mybir
import concourse.tile as tile
from concourse.tile_rust import add_dep_helper
from concourse.bass_utils import run_bass_kernel_spmd

NCORES = 8
B, C, D, H0, H1 = 16384, 8, 256, 512, 256
BS = B // NCORES          # 2048 batch rows per core
BC = 512                  # batch chunk (one psum bank of fp32)
NB = BS // BC             # 4
KT0, MT0 = D // 128, H0 // 128    # 2, 4
KT1, MT1 = H0 // 128, H1 // 128   # 4, 2
KT2 = H1 // 128                   # 2

WCOLS = KT0 * H0 + KT1 * H1       # per-c combined weight columns (w0 then w1)
W2_COLS = C * KT2 * C             # 128
B0_OFF, B1_OFF, B2_OFF = 0, C * MT0, C * MT0 + C * MT1
BIAS_COLS = B2_OFF + 1            # 49

# matmul operand mode: "f32" (exact, 4 cyc/row), "f32r" (reduced-precision
# fp32 path, 1 cyc/row at N>=256), "bf16" (1 cyc/row, halves DMA traffic)
MM_MODE = os.environ.get("KERNEL_MM_MODE", "f32r")

_CACHE = {}


def _build(mm: str) -> bass.Bass:
    f32 = mybir.dt.float32
    if mm == "bf16":
        wdt = mybir.dt.bfloat16
    elif mm == "f32r":
        wdt = mybir.dt.float32r
    else:
        wdt = f32
    relu = mybir.ActivationFunctionType.Relu
    copy_f = mybir.ActivationFunctionType.Copy
    add_op = mybir.AluOpType.add
    max_op = mybir.AluOpType.max

    nc = bass.Bass(trn_type="TRN2")
    xd = nc.dram_tensor("xd", [128, NB, KT0, BC], wdt, kind="ExternalInput")
    w2d = nc.dram_tensor("w2d", [128, W2_COLS], wdt, kind="ExternalInput")
    wcat = nc.dram_tensor("wcat", [C, 128, WCOLS], wdt, kind="ExternalInput")
    biasd = nc.dram_tensor("biasd", [128, BIAS_COLS], f32, kind="ExternalInput")
    outT = nc.dram_tensor("outT", [C, BS], f32, kind="ExternalOutput")

    with tile.TileContext(nc) as tc, ExitStack() as ctx:
        consts = ctx.enter_context(tc.tile_pool(name="consts", bufs=1))
        h0p = ctx.enter_context(tc.tile_pool(name="h0", bufs=2))
        h1p = ctx.enter_context(tc.tile_pool(name="h1", bufs=2))
        ps0p = ctx.enter_context(tc.tile_pool(name="ps0", bufs=3, space="PSUM"))
        ps1p = ctx.enter_context(tc.tile_pool(name="ps1", bufs=2, space="PSUM"))
        ps2p = ctx.enter_context(tc.tile_pool(name="ps2", bufs=2, space="PSUM"))
        pewp = ctx.enter_context(tc.tile_pool(name="pew", bufs=1, space="PSUM"))

        # HWDGE DMAs drain FIFO per ring, so issue in consumption order:
        # the small w2/bias blocks, then x and per-c weights interleaved so
        # the first layer-0 matmul only waits ~1.3 MB, and each c's weights
        # land while the previous c computes.
        funnel_deps = []
        w2s = consts.tile([128, W2_COLS], wdt, tag="w2s")
        funnel_deps.append(nc.sync.dma_start(out=w2s, in_=w2d[:, :]))
        bias = consts.tile([128, BIAS_COLS], f32, tag="bias")
        funnel_deps.append(nc.sync.dma_start(out=bias, in_=biasd[:, :]))
        xs = consts.tile([128, NB, KT0, BC], wdt, tag="xs")
        wts = []
        for c in range(C):
            wc_tile = consts.tile([128, WCOLS], wdt, tag=f"w_{c}")
            wts.append(wc_tile)
        W0C = KT0 * H0  # first WCOLS half = layer-0 weights, rest = layer-1
        dma_order = [("x", 0), ("w0", 0), ("w1", 0), ("w0", 1), ("w1", 1),
                     ("x", 1), ("w0", 2), ("w1", 2), ("w0", 3), ("w1", 3),
                     ("x", 2), ("w0", 4), ("w1", 4), ("w0", 5), ("w1", 5),
                     ("x", 3), ("w0", 6), ("w1", 6), ("w0", 7), ("w1", 7)]
        for kind, i in dma_order:
            eng = nc.sync
            if kind == "x":
                funnel_deps.append(
                    eng.dma_start(out=xs[:, i], in_=xd[:, i])
                )
            elif kind == "w0":
                funnel_deps.append(
                    eng.dma_start(out=wts[i][:, 0:W0C], in_=wcat[i][:, 0:W0C])
                )
            else:
                funnel_deps.append(
                    eng.dma_start(
                        out=wts[i][:, W0C:WCOLS], in_=wcat[i][:, W0C:WCOLS]
                    )
                )

        def xs_ap(kt, bi):
            return xs[:, bi, kt, :]

        def w2_ap(t):
            return w2s[:, t * C:(t + 1) * C]

        def w0_ap(c, kt, mt):
            lo = kt * H0 + mt * 128
            return wts[c][:, lo:lo + 128]

        def w1_ap(c, kt, mt):
            lo = KT0 * H0 + kt * H1 + mt * 128
            return wts[c][:, lo:lo + 128]

        def b0_ap(c, mt):
            return bias[:, B0_OFF + c * MT0 + mt:B0_OFF + c * MT0 + mt + 1]

        def b1_ap(c, mt):
            return bias[:, B1_OFF + c * MT1 + mt:B1_OFF + c * MT1 + mt + 1]

        b2_ap = bias[0:C, B2_OFF:B2_OFF + 1]

        # Per-engine scratch: unique column per touch so no touch ever
        # overlaps another (no WAW deps between touches at all).
        act_scr = consts.tile([1, 256], f32, tag="act_scr")
        dve_scr = consts.tile([1, 256], f32, tag="dve_scr")
        outt = consts.tile([C, NB, BC], f32, tag="outt")
        pewarm = pewp.tile([C, BC], f32, tag="pewarm")
        cnt = {"act": 0, "dve": 0, "pe": 0}

        def act_touch(src_ap):
            j = cnt["act"]; cnt["act"] += 1
            return nc.scalar.activation(act_scr[0:1, j:j + 1], src_ap, copy_f)

        def dve_touch(src_ap):
            j = cnt["dve"]; cnt["dve"] += 1
            return nc.vector.tensor_copy(dve_scr[0:1, j:j + 1], src_ap)

        def pe_touch(col_ap):
            # col_ap: [P, 1] SBUF column; writes 1 garbage element into the
            # never-read pewarm bank. Absorbs exactly one sem wait on PE.
            # f32r APs are bitcast to f32: tiny-N matmuls trip the ISA's
            # fp32r restrictions, and the result is garbage anyway.
            j = cnt["pe"]; cnt["pe"] += 1
            if col_ap.dtype == mybir.dt.float32r:
                col_ap = col_ap.bitcast(mybir.dt.float32)
            return nc.tensor.matmul(
                pewarm[0:1, j:j + 1], lhsT=col_ap, rhs=col_ap,
                start=True, stop=True,
            )

        # Pre-observe the bias DMA lane on both evict engines, and the w2
        # DMA lane on PE (the opener consumes w2s first).
        act_bias_touch = act_touch(bias[0:1, 0:1])
        dve_bias_touch = dve_touch(bias[0:1, 0:1])
        pe_w2s_touch = pe_touch(w2s[:, 0:1])

        first_evict = {"act": True, "dve": True}
        last_inst = {"pe": None, "dve": None}

        def evict(engine, dst_ap, src_ps, bias_ap, with_relu=True):
            # Touch first (absorbs the PE-completion wait), then the real
            # evict carries only its slot-release self-wait.
            if engine == "act":
                t = act_touch(src_ps[0:1, 0:1])
                if with_relu:
                    e = nc.scalar.activation(dst_ap, src_ps, relu, bias=bias_ap)
                else:
                    e = nc.scalar.add(dst_ap, src_ps, bias_ap)
            else:
                t = dve_touch(src_ps[0:1, 0:1])
                e = nc.vector.tensor_scalar(
                    dst_ap, src_ps, bias_ap, 0.0, add_op, max_op
                )
            add_dep_helper(e.ins, t.ins, False, "touch before evict")
            if engine == "dve":
                last_inst["dve"] = e
            if first_evict[engine]:
                first_evict[engine] = False
                bt = act_bias_touch if engine == "act" else dve_bias_touch
                add_dep_helper(t.ins, bt.ins, False, "bias touch first")
            return e

        for bi in range(NB):
            if bi >= 2:
                # PE re-acquires the ps2 psum slot: observe the ACT
                # out-evict that last read it, then let the opener matmul
                # absorb the slot-release wait.
                ot = pe_touch(outt[:, bi - 2, 0:1])
            ps2 = ps2p.tile([C, BC], f32)
            op_w = w2_ap(0)
            if op_w.dtype == mybir.dt.float32r:
                op_w = op_w.bitcast(mybir.dt.float32)
            opener = nc.tensor.matmul(
                ps2[:, 0:C], lhsT=op_w, rhs=op_w,
                start=True, stop=True,
            )
            add_dep_helper(opener.ins, (ot if bi >= 2 else pe_w2s_touch).ins,
                           False, "observe before opener")
            # Observe this bi's x-chunk DMA lane before its first matmul.
            xt = pe_touch(xs[:, bi, 0, 0:1])
            add_dep_helper(xt.ins, opener.ins, False, "opener before x touch")
            for c in range(C):
                e0, e1 = ("act", "dve") if c % 2 == 0 else ("dve", "act")
                if bi == 0:
                    wt = pe_touch(wts[c][:, 0:1])
                h0 = h0p.tile([128, KT1, BC], wdt)
                for mt in range(MT0):
                    ps0 = ps0p.tile([128, BC], f32)
                    for kt in range(KT0):
                        mmi = nc.tensor.matmul(
                            ps0,
                            lhsT=w0_ap(c, kt, mt),
                            rhs=xs_ap(kt, bi),
                            start=(kt == 0),
                            stop=(kt == KT0 - 1),
                        )
                        if bi == 0 and mt == 0 and kt == 0:
                            add_dep_helper(mmi.ins, wt.ins, False, "wt touch first")
                        if c == 0 and mt == 0 and kt == 0:
                            add_dep_helper(mmi.ins, xt.ins, False, "x touch first")
                    evict(e0, h0[:, mt, :], ps0, b0_ap(c, mt))
                if bi == 0:
                    wt1 = pe_touch(wts[c][:, W0C:W0C + 1])
                h1 = h1p.tile([128, KT2, BC], wdt)
                for mt in range(MT1):
                    ps1 = ps1p.tile([128, BC], f32)
                    for kt in range(KT1):
                        mm1 = nc.tensor.matmul(
                            ps1,
                            lhsT=w1_ap(c, kt, mt),
                            rhs=h0[:, kt, :],
                            start=(kt == 0),
                            stop=(kt == KT1 - 1),
                        )
                        if bi == 0 and mt == 0 and kt == 0:
                            add_dep_helper(mm1.ins, wt1.ins, False, "w1 touch first")
                    evict(e1, h1[:, mt, :], ps1, b1_ap(c, mt))
                for kt in range(KT2):
                    last_inst["pe"] = nc.tensor.matmul(
                        ps2,
                        lhsT=w2_ap(c * KT2 + kt),
                        rhs=h1[:, kt, :],
                        start=(c == 0 and kt == 0),
                        stop=(c == C - 1 and kt == KT2 - 1),
                    )
            out_ev = evict("act", outt[:, bi, :], ps2, b2_ap, with_relu=False)
        # Single output DMA on the (otherwise empty) SWDGE ring: one wait
        # (ACT wrote outt) and one extra drain sem total.
        out_dma = nc.gpsimd.dma_start(
            out=outT[:, :], in_=outt.rearrange("c nb bc -> c (nb bc)")
        )
        # Funnel: the kernel-tail drain would otherwise wait on every DMA
        # lane + engine sem at once (way over the 1-wait-per-instruction
        # limit). A chain of single-wait SP nops observes each outstanding
        # sem first, leaving the drain nothing new to wait on.
        funnel_deps += [out_dma, out_ev, last_inst["pe"], last_inst["dve"]]
        for dep in funnel_deps:
            n = nc.sync.nop()
            add_dep_helper(n.ins, dep.ins, True, "drain funnel")
    return nc


def _np_wdt(mm: str):
    if mm == "bf16":
        import ml_dtypes

        return ml_dtypes.bfloat16
    return np.float32


def kernel(x, W0, b0, W1, b1, W2, b2, trace=False):
    mm = MM_MODE
    key = ("nc", mm)
    if key not in _CACHE:
        _CACHE[key] = _build(mm)
    nc = _CACHE[key]
    wnp = _np_wdt(mm)

    x = np.ascontiguousarray(np.asarray(x, dtype=np.float32))
    W0 = np.asarray(W0, dtype=np.float32)
    W1 = np.asarray(W1, dtype=np.float32)
    W2 = np.asarray(W2, dtype=np.float32)
    b0 = np.asarray(b0, dtype=np.float32)
    b1 = np.asarray(b1, dtype=np.float32)
    b2 = np.asarray(b2, dtype=np.float32)

    # Combined per-c weight block: [C, 128, KT0*H0 + KT1*H1] where
    # wcat[c, p, kt*H0 + h] = W0[c, h, kt*128+p] and
    # wcat[c, p, KT0*H0 + kt*H1 + o] = W1[c, o, kt*128+p].
    wcat = np.empty((C, 128, WCOLS), dtype=np.float32)
    w0v = wcat[:, :, :KT0 * H0].reshape(C, 128, KT0, H0)
    w0v[...] = W0.reshape(C, H0, KT0, 128).transpose(0, 3, 2, 1)
    w1v = wcat[:, :, KT0 * H0:].reshape(C, 128, KT1, H1)
    w1v[...] = W1.reshape(C, H1, KT1, 128).transpose(0, 3, 2, 1)
    wcat = np.ascontiguousarray(wcat).astype(wnp)

    # Layer-2 block-diagonal lhsT tiles:
    # w2part[p, t*C + cc] = W2[c, 0, i] iff (c, i) == (t // KT2, (t % KT2)*128 + p)
    w2part = np.zeros((128, W2_COLS), dtype=np.float32)
    for c in range(C):
        for kt in range(KT2):
            t = c * KT2 + kt
            w2part[:, t * C + c] = W2[c, 0, kt * 128:(kt + 1) * 128]
    w2part = w2part.astype(wnp)

    biasd = np.zeros((128, BIAS_COLS), dtype=np.float32)
    biasd[:, B0_OFF:B0_OFF + C * MT0] = (
        b0.reshape(C, MT0, 128).transpose(2, 0, 1).reshape(128, C * MT0)
    )
    biasd[:, B1_OFF:B1_OFF + C * MT1] = (
        b1.reshape(C, MT1, 128).transpose(2, 0, 1).reshape(128, C * MT1)
    )
    biasd[0:C, B2_OFF] = b2

    xTfull = np.ascontiguousarray(x.T)  # [D, B] fp32
    in_maps = []
    for s in range(NCORES):
        xsh = xTfull[:, s * BS:(s + 1) * BS]          # [D, BS]
        # xd[p, bi, kt, b] = x[s*BS + bi*BC + b, kt*128 + p]
        xd = np.ascontiguousarray(
            xsh.reshape(KT0, 128, NB, BC).transpose(1, 2, 0, 3)
        ).astype(wnp)
        in_maps.append(
            {
                "xd": xd,
                "w2d": w2part,
                "wcat": wcat,
                "biasd": biasd,
            }
        )

    res = run_bass_kernel_spmd(
        nc, in_maps, core_ids=list(range(NCORES)), trace=trace
    )
    _CACHE["last_result"] = res

    out = np.empty((B, C), dtype=np.float32)
    for s in range(NCORES):
        out[s * BS:(s + 1) * BS, :] = res.results[s]["outT"].T
    return out
